# revision 15
# baseline (speedup 1.0000x reference)
"""CrossAttention Trainium2 kernel (8 NeuronCores), v2.

Reference computation (B=2, N=M=2048, D=1024, H=16, C=64):
    q = rmsnorm(querys @ Wq.T, gq) * C**-0.5       [B,N,D]
    k = rmsnorm(key_feats @ Wk.T, gk)              [B,M,D]
    v = key_feats @ Wv.T                           [B,M,D]
    attn = softmax(mask(q @ k.T per head))         [B,H,N,M]
    out = (attn @ v per head, concat) @ Wo.T + bo  [B,N,D]

Sharding: core = b*4 + j (b in {0,1}; j owns heads 4j..4j+3 = a 256-wide
d-slice). v2 changes vs v1:

  - Mask compaction: the host packs only mask==1 key rows (plus zero pad
    to Mp, a multiple of 128). Attention + k/v projections shrink ~2x.
    Pad rows are killed by the -1e30 exp bias.
  - No collectives. The full-D rmsnorm sum-of-squares is computed locally
    per core via a host-side Cholesky factor: sumsq(x) = ||L^T x||^2 with
    G = W_raw^T W_raw = L L^T. L is block-lower-triangular, so only 36 of
    64 [128,128] blocks contribute. ss = colsum(z^2) via DVE square + PE
    ones-column matmul. This removes the AllReduce bootstrap (~110us) +
    latency (~70us) and the PE idle window that collapsed the HAM PE
    clock to 4/8 for the whole attention phase.
  - kfT streamed once: k-proj, z_k and v-proj all consume the same SBUF
    block. Input DMA spread over sync/gpsimd/scalar queues.
  - Out-projection interleaved into the attention instruction stream
    (one ot-chunk per m-tile) once an n-block's xT is finalized; only the
    last two n-blocks drain after attention.
"""

import numpy as np

import concourse.tile as tile
from concourse import bacc, mybir
from concourse.bass_utils import run_bass_kernel_spmd

B, N, M, D, H = 2, 2048, 2048, 1024, 16
C = D // H  # 64, head dim
E = D  # input feature dim
EPS = 1e-6
SCALE = C ** (-0.5)
DS = D // 4  # 256, per-core d-slice
NCORES = 8

f32 = mybir.dt.float32
f32r = mybir.dt.float32r
bf16 = mybir.dt.bfloat16
AF = mybir.ActivationFunctionType

NEG = -1e30

# block-lower-triangle of L in [e, zd] 128-blocks: L[e, zd] != 0 for zd <= e
TRI = [(dz, et) for dz in range(8) for et in range(dz, 8)]
NTRI = len(TRI)  # 36


def round_f32r(x: np.ndarray) -> np.ndarray:
    b = np.ascontiguousarray(x, dtype=np.float32).view(np.uint32)
    b = (b + 0x800) & np.uint32(0xFFFFF000)
    return b.view(np.float32)


def build(Mp: int):
    MT = Mp // 128
    # kf stream blocks of up to 512 columns
    W_LIST = [(s, min(512, Mp - s)) for s in range(0, Mp, 512)]

    nc = bacc.Bacc(None, target_bir_lowering=False)

    qT_d = nc.declare_dram_parameter("qT", [E, N], f32r, isOutput=False)
    kfT_d = nc.declare_dram_parameter("kfT", [E, Mp], f32r, isOutput=False)
    wqT_d = nc.declare_dram_parameter("wqT", [E, DS], f32r, isOutput=False)
    wkT_d = nc.declare_dram_parameter("wkT", [E, DS], f32r, isOutput=False)
    wvT_d = nc.declare_dram_parameter("wvT", [E, DS], f32r, isOutput=False)
    woT_d = nc.declare_dram_parameter("woT", [DS, D], f32r, isOutput=False)
    lq_d = nc.declare_dram_parameter("Lq", [128, NTRI * 128], f32r, isOutput=False)
    lk_d = nc.declare_dram_parameter("Lk", [128, NTRI * 128], f32r, isOutput=False)
    mb_d = nc.declare_dram_parameter("mbias", [MT, 128], f32, isOutput=False)
    outT_d = nc.declare_dram_parameter("outT", [D, N], bf16, isOutput=True)

    with (
        nc.allow_low_precision(reason="f32r matmul operands by design; fp32 PSUM"),
        tile.TileContext(nc) as tc,
    ):
        with (
            tc.tile_pool(name="singles", bufs=1) as singles,
            tc.tile_pool(name="wts", bufs=3) as wts,
            tc.tile_pool(name="lw", bufs=2) as lpool,
            tc.tile_pool(name="blk", bufs=3) as blkpool,
            tc.tile_pool(name="sq", bufs=2) as sqpool,
            tc.tile_pool(name="psb", bufs=3) as ppool,
            tc.tile_pool(name="obuf", bufs=2) as obuf,
            tc.tile_pool(name="osb", bufs=2) as osbp,
            tc.tile_pool(name="rdp", bufs=4) as rdp,
            tc.tile_pool(name="small", bufs=4) as small,
            tc.tile_pool(name="dram", bufs=1, space="DRAM") as dram,
        ):
            # round-robin input-stream DMA queues (PE untouched; DVE busy)
            inq = [nc.sync, nc.gpsimd, nc.scalar]
            qn = [0]

            def dq():
                e = inq[qn[0] % 3]
                qn[0] += 1
                return e

            outq_eng = [nc.sync, nc.scalar, nc.gpsimd]

            # ---- constants / small inputs ----
            ones_f = singles.tile([128, 64], f32)
            nc.vector.memset(ones_f, 1.0)
            ones1x64 = singles.tile([1, 64], f32)
            nc.vector.memset(ones1x64, 1.0)
            ones1x128 = singles.tile([1, 128], f32)
            nc.vector.memset(ones1x128, 1.0)
            ones_col = singles.tile([128, 1], f32r)
            nc.vector.tensor_copy(ones_col, ones_f[:, 0:1])
            eps_t = singles.tile([128, 1], f32)
            nc.vector.memset(eps_t, EPS)
            invd_t = singles.tile([128, 1], f32)
            nc.vector.memset(invd_t, 1.0 / D)
            mb_sb = singles.tile([128, MT], f32)
            nc.gpsimd.dma_start(out=mb_sb, in_=mb_d.rearrange("t p -> p t"))

            # ---- persistent activations ----
            qT = singles.tile([128, 2, 4, 512], f32r)  # [p, dt, nb, n]
            kT = singles.tile([128, 2, MT, 128], f32r)  # [p, dt, mt, m]
            v_sb = singles.tile([128, MT, 4, C + 1], f32r)  # [m_p, mt, h, c|ones]
            xT = qT  # aliased: each [h, nb] slice is written only after its last QK read
            nc.vector.tensor_copy(
                v_sb[:, :, :, C],
                ones_f[:, 0:MT * 4].rearrange("p (a b) -> p a b", a=MT),
            )
            ssq_row = singles.tile([1, 2048], f32)
            ssk_row = singles.tile([1, Mp], f32)

            # ---- weights (consumption order) ----
            wq_sb = wts.tile([128, 8, DS], f32r, tag="w")
            for et in range(8):
                dq().dma_start(out=wq_sb[:, et, :], in_=wqT_d[et * 128 : et * 128 + 128, :])
            lq_sb = lpool.tile([128, NTRI, 128], f32r, tag="L")
            for c in range(4):
                dq().dma_start(
                    out=lq_sb[:, c * 9 : c * 9 + 9, :],
                    in_=lq_d[:, c * 9 * 128 : (c * 9 + 9) * 128].rearrange(
                        "p (a b) -> p a b", a=9
                    ),
                )
            LIDX = {b: i for i, b in enumerate(TRI)}

            with (
                tc.tile_pool(name="projps", bufs=2, space="PSUM") as projps,
                tc.tile_pool(name="zps", bufs=2, space="PSUM") as zps,
                tc.tile_pool(name="ssps", bufs=2, space="PSUM") as ssps,
                tc.tile_pool(name="vps", bufs=2, space="PSUM") as vps,
            ):
                def emit_z(blk, l_sb, ss_seg, w):
                    """ss_seg[1, w] += colsum over full zd of (L^T x)^2."""
                    ss_ps = ssps.tile([1, 512], f32, tag="ss")
                    pend = []

                    def colsum(dz, zp):
                        sq = sqpool.tile([128, 512], f32r, tag="sq")
                        nc.scalar.activation(sq[:, 0:w], zp[:, 0:w], AF.Square)
                        nc.tensor.matmul(
                            ss_ps[:, 0:w],
                            ones_col,
                            sq[:, 0:w],
                            start=(dz == 0),
                            stop=(dz == 7),
                            skip_group_check=True,
                        )

                    for dz in range(8):
                        zp = zps.tile([128, 512], f32, tag="z")
                        for et in range(dz, 8):
                            nc.tensor.matmul(
                                zp[:, 0:w],
                                l_sb[:, LIDX[(dz, et)], :],
                                blk[:, et, 0:w],
                                start=(et == dz),
                                stop=(et == 7),
                            )
                        pend.append((dz, zp))
                        if len(pend) > 1:
                            colsum(*pend.pop(0))
                    colsum(*pend.pop(0))
                    nc.vector.tensor_copy(ss_seg, ss_ps[:, 0:w])

                # ---- P1: q projection + z_q, z staggered one block behind ----
                pend_z = []
                for nb in range(4):
                    blk = blkpool.tile([128, 8, 512], f32r, tag="blk")
                    for et in range(8):
                        dq().dma_start(
                            out=blk[:, et, :],
                            in_=qT_d[et * 128 : et * 128 + 128, nb * 512 : nb * 512 + 512],
                        )
                    for dt in range(2):
                        ps = projps.tile([128, 512], f32, tag="proj")
                        for et in range(8):
                            nc.tensor.matmul(
                                ps,
                                wq_sb[:, et, dt * 128 : dt * 128 + 128],
                                blk[:, et, :],
                                start=(et == 0),
                                stop=(et == 7),
                            )
                        nc.vector.tensor_copy(qT[:, dt, nb, :], ps)
                    pend_z.append((blk, nb))
                    if nb == 1:
                        # k weights + first kf block next in queue order
                        wk_sb = wts.tile([128, 8, DS], f32r, tag="w")
                        for et in range(8):
                            dq().dma_start(
                                out=wk_sb[:, et, :],
                                in_=wkT_d[et * 128 : et * 128 + 128, :],
                            )
                    if len(pend_z) > 2:
                        b0, n0 = pend_z.pop(0)
                        emit_z(b0, lq_sb, ssq_row[:, n0 * 512 : n0 * 512 + 512], 512)
                while pend_z:
                    b0, n0 = pend_z.pop(0)
                    emit_z(b0, lq_sb, ssq_row[:, n0 * 512 : n0 * 512 + 512], 512)

                # ---- rstd_q + qT finalize ----
                nc.scalar.activation(
                    ssq_row, ssq_row, AF.Sqrt, bias=eps_t[0:1, :], scale=invd_t[0:1, :]
                )
                rs_row = singles.tile([1, 2048], f32)
                nc.vector.reciprocal_approx_fast(out=rs_row, in_=ssq_row)
                for nb in range(4):
                    bcq = projps.tile([128, 512], f32, tag="proj")
                    nc.tensor.matmul(
                        bcq,
                        ones1x128,
                        rs_row[:, nb * 512 : nb * 512 + 512],
                        start=True,
                        stop=True,
                    )
                    for dt in range(2):
                        nc.vector.tensor_mul(qT[:, dt, nb, :], qT[:, dt, nb, :], bcq)

                # ---- P2: k proj + z_k + v proj, one kf stream ----
                lk_sb = lpool.tile([128, NTRI, 128], f32r, tag="L")
                wv_sb = wts.tile([128, 8, DS], f32r, tag="w")
                for bi, (s0, w) in enumerate(W_LIST):
                    blk = blkpool.tile([128, 8, 512], f32r, tag="blk")
                    for et in range(8):
                        dq().dma_start(
                            out=blk[:, et, 0:w],
                            in_=kfT_d[et * 128 : et * 128 + 128, s0 : s0 + w],
                        )
                    if bi == 0:
                        for c in range(4):
                            dq().dma_start(
                                out=lk_sb[:, c * 9 : c * 9 + 9, :],
                                in_=lk_d[:, c * 9 * 128 : (c * 9 + 9) * 128].rearrange(
                                    "p (a b) -> p a b", a=9
                                ),
                            )
                        for et in range(8):
                            dq().dma_start(
                                out=wv_sb[:, et, :],
                                in_=wvT_d[et * 128 : et * 128 + 128, :],
                            )
                    nmt = w // 128
                    for dt in range(2):
                        ps = projps.tile([128, 512], f32, tag="proj")
                        for et in range(8):
                            nc.tensor.matmul(
                                ps[:, 0:w],
                                wk_sb[:, et, dt * 128 : dt * 128 + 128],
                                blk[:, et, 0:w],
                                start=(et == 0),
                                stop=(et == 7),
                            )
                        nc.vector.tensor_copy(
                            kT[:, dt, bi * 4 : bi * 4 + nmt, :],
                            ps[:, 0:w].rearrange("p (a b) -> p a b", a=nmt),
                        )
                    emit_z(blk, lk_sb, ssk_row[:, s0 : s0 + w], w)
                    for mtL in range(nmt):
                        psv = vps.tile([128, 256], f32, tag="v")
                        for et in range(8):
                            nc.tensor.matmul(
                                psv,
                                blk[:, et, mtL * 128 : mtL * 128 + 128],
                                wv_sb[:, et, :],
                                start=(et == 0),
                                stop=(et == 7),
                            )
                        nc.vector.tensor_copy(
                            v_sb[:, bi * 4 + mtL, :, 0:C],
                            psv.rearrange("p (h c) -> p h c", c=C),
                        )

                # wo load (consumed mid-attention)
                wo_sb = wts.tile([128, 2, D], f32r, tag="w")
                for dc in range(2):
                    dq().dma_start(
                        out=wo_sb[:, dc, :], in_=woT_d[dc * 128 : dc * 128 + 128, :]
                    )

                # ---- rstd_k in row layout, folded into kT via bcast muls ----
                nc.scalar.activation(
                    ssk_row, ssk_row, AF.Sqrt, bias=eps_t[0:1, :], scale=invd_t[0:1, :]
                )
                rk_row = singles.tile([1, Mp], f32)
                nc.vector.reciprocal_approx_fast(out=rk_row, in_=ssk_row)
                for mt in range(MT):
                    bck = projps.tile([128, 512], f32, tag="proj")
                    nc.tensor.matmul(
                        bck[:, 0:128],
                        ones1x128,
                        rk_row[:, mt * 128 : mt * 128 + 128],
                        start=True,
                        stop=True,
                    )
                    for dt in range(2):
                        nc.vector.tensor_mul(
                            kT[:, dt, mt, :], kT[:, dt, mt, :], bck[:, 0:128]
                        )

            # ---- P4: attention, nbp-outer, out-proj interleaved ----
            with (
                tc.tile_pool(name="sps", bufs=2, space="PSUM") as spool,
                tc.tile_pool(name="ops", bufs=1, space="PSUM") as opool,
                tc.tile_pool(name="dmy", bufs=1, space="PSUM") as dmypool,
            ):
                dum = dmypool.tile([128, 512], f32, tag="dum")

                def emit_outproj_chunk(nb, ot):
                    ps2 = spool.tile([128, 2, 512], f32, tag="s2")
                    ps = ps2[:, 0, :]
                    for dc in range(2):
                        nc.tensor.matmul(
                            ps,
                            wo_sb[:, dc, ot * 128 : ot * 128 + 128],
                            xT[:, dc, nb, :],
                            start=(dc == 0),
                            stop=(dc == 1),
                            skip_group_check=True,
                        )
                    out_sb = osbp.tile([128, 512], bf16, tag="osb")
                    nc.vector.tensor_copy(out_sb, ps)
                    outq_eng[(nb * 8 + ot) % 3].dma_start(
                        out=outT_d[ot * 128 : ot * 128 + 128, nb * 512 : nb * 512 + 512],
                        in_=out_sb,
                    )

                def emit_normalize(state):
                    """bc outer-products + muls for a pass whose DVE recips are
                    done by now (emitted one pass late to keep PE gapless)."""
                    hh, nbp, oo_sb, rds = state
                    ddt, ooff = hh // 2, (hh % 2) * C
                    for i, nb in enumerate((2 * nbp, 2 * nbp + 1)):
                        bc = spool.tile([128, 2, 512], f32, tag="s2")
                        nc.tensor.matmul(
                            bc[0:C, 0, :], ones1x64, rds[i], start=True, stop=True
                        )
                        nc.vector.tensor_mul(
                            xT[ooff : ooff + C, ddt, nb, :],
                            oo_sb[0:C, i, :],
                            bc[0:C, 0, :],
                        )

                prev = None
                for nbp in range(2):
                    for h in range(4):
                        dt, off = h // 2, (h % 2) * C
                        nbs = (2 * nbp, 2 * nbp + 1)
                        o2 = opool.tile([C + 1, 2, 512], f32, tag="o2")
                        for mt in range(MT):
                            kT_lhs = kT[off : off + C, dt, mt, :]
                            s2 = spool.tile([128, 2, 512], f32, tag="s2")
                            for i, nb in enumerate(nbs):
                                nc.tensor.matmul(
                                    s2[:, i, :],
                                    kT_lhs,
                                    qT[off : off + C, dt, nb, :],
                                    start=True,
                                    stop=True,
                                )
                            p2 = ppool.tile([128, 2, 512], f32r, tag="p")
                            nc.scalar.activation(
                                p2, s2, AF.Exp, bias=mb_sb[:, mt : mt + 1]
                            )
                            for i in range(2):
                                nc.tensor.matmul(
                                    o2[:, i, :],
                                    v_sb[:, mt, h, :],
                                    p2[:, i, :],
                                    start=(mt == 0),
                                    stop=(mt == MT - 1),
                                    skip_group_check=True,
                                )
                            # pad PE to ~98% of the ACT exp pace so the HAM
                            # activity monitor keeps the PE clock at 8/8
                            for wd in (4, 3):
                                nc.tensor.matmul(
                                    dum[:, 0 : wd * 128],
                                    kT[:, 0, 0, :],
                                    kT[:, 0, 0:wd, :],
                                    start=True, stop=True,
                                    skip_group_check=True,
                                )
                        o_sb = obuf.tile([C + 1, 2, 512], f32, tag="osb")
                        nc.vector.tensor_copy(o_sb, o2)
                        rds = []
                        for i in range(2):
                            den_sb = rdp.tile([1, 512], f32, tag="den")
                            nc.vector.tensor_copy(den_sb, o_sb[C : C + 1, i, :])
                            rd = rdp.tile([1, 512], f32, tag="rd")
                            nc.vector.reciprocal_approx_fast(out=rd, in_=den_sb)
                            rds.append(rd)
                        if prev is not None:
                            emit_normalize(prev)
                        prev = (h, nbp, o_sb, rds)
                emit_normalize(prev)
                for nb in range(4):
                    for ot in range(8):
                        emit_outproj_chunk(nb, ot)
                dum_sink = rdp.tile([1, 512], f32, tag="rd")
                nc.vector.tensor_copy(dum_sink, dum[0:1, :])

    nc.finalize()
    return nc


_NC_CACHE = {}


def _get_nc(Mp=1024):
    if Mp not in _NC_CACHE:
        _NC_CACHE[Mp] = build(Mp)
    return _NC_CACHE[Mp]


def _chol_factor(W):
    G = W.astype(np.float64).T @ W.astype(np.float64)
    G += np.eye(E) * (1e-12 * np.trace(G) / E)
    L = np.linalg.cholesky(G)
    return L.astype(np.float32)  # [e, zd] lower


def _pack_L(L):
    P = np.empty((128, NTRI * 128), np.float32)
    for bi, (dz, et) in enumerate(TRI):
        P[:, bi * 128 : (bi + 1) * 128] = L[
            et * 128 : (et + 1) * 128, dz * 128 : (dz + 1) * 128
        ]
    return round_f32r(P)


def plan_Mp(mask):
    mask = np.asarray(mask)
    Mv = [int((mask[b] != 0).sum()) for b in range(B)]
    Mp = max(128, int(-(-max(max(Mv), 1) // 128)) * 128)
    return Mv, Mp


def make_in_maps(querys, key_feats, mask, Wq, Wk, Wv, gq, gk, Wo, bo):
    querys = np.asarray(querys, dtype=np.float32)
    key_feats = np.asarray(key_feats, dtype=np.float32)
    mask = np.asarray(mask)
    gq = np.asarray(gq, dtype=np.float32)
    gk = np.asarray(gk, dtype=np.float32)
    Wq = np.asarray(Wq, dtype=np.float32)
    Wk = np.asarray(Wk, dtype=np.float32)

    Mv, Mp = plan_Mp(mask)
    MT = Mp // 128

    gsq_full = gq * np.float32(SCALE)
    Wq_f = Wq * gsq_full[:, None]
    Wk_f = Wk * gk[:, None]
    lq_p = _pack_L(_chol_factor(Wq))
    lk_p = _pack_L(_chol_factor(Wk))

    qT, kfT, mb = [], [], []
    for b in range(B):
        idx = np.nonzero(mask[b])[0]
        kfc = np.zeros((Mp, E), np.float32)
        kfc[: len(idx)] = key_feats[b][idx]
        mbias = np.full((MT, 128), np.float32(NEG), np.float32)
        mbias.reshape(-1)[: len(idx)] = 0.0
        qT.append(round_f32r(querys[b].T))
        kfT.append(round_f32r(kfc.T))
        mb.append(mbias)

    wqT, wkT, wvT, woT = [], [], [], []
    for j in range(4):
        dsl = slice(j * DS, (j + 1) * DS)
        wqT.append(round_f32r(Wq_f[dsl].T))
        wkT.append(round_f32r(Wk_f[dsl].T))
        wvT.append(round_f32r(np.asarray(Wv)[dsl].T))
        woT.append(round_f32r(np.asarray(Wo)[:, dsl].T))

    in_maps = []
    for cid in range(NCORES):
        b, j = cid // 4, cid % 4
        in_maps.append(
            {
                "qT": qT[b],
                "kfT": kfT[b],
                "wqT": wqT[j],
                "wkT": wkT[j],
                "wvT": wvT[j],
                "woT": woT[j],
                "Lq": lq_p,
                "Lk": lk_p,
                "mbias": mb[b],
            }
        )
    return in_maps


def assemble(results, mask, bo):
    mask = np.asarray(mask)
    bo = np.asarray(bo, dtype=np.float32)
    out = np.zeros((B, N, D), dtype=np.float32)
    for cid in range(NCORES):
        b = cid // 4
        out[b] += np.asarray(results[cid]["outT"]).astype(np.float32).T
    out += bo
    for b in range(B):
        if (mask[b] != 0).sum() == 0:
            out[b] = bo  # reference: all-masked row -> attn = 0
    return out


def kernel(querys, key_feats, mask, Wq, Wk, Wv, gq, gk, Wo, bo):
    _, Mp = plan_Mp(mask)
    nc = _get_nc(Mp)
    in_maps = make_in_maps(querys, key_feats, mask, Wq, Wk, Wv, gq, gk, Wo, bo)
    res = run_bass_kernel_spmd(nc, in_maps, list(range(NCORES)))
    return assemble(res.results, mask, bo)


# revision 17
# speedup vs baseline: 1.1757x; 1.1757x over previous
"""CrossAttention Trainium2 kernel (8 NeuronCores), v2.

Reference computation (B=2, N=M=2048, D=1024, H=16, C=64):
    q = rmsnorm(querys @ Wq.T, gq) * C**-0.5       [B,N,D]
    k = rmsnorm(key_feats @ Wk.T, gk)              [B,M,D]
    v = key_feats @ Wv.T                           [B,M,D]
    attn = softmax(mask(q @ k.T per head))         [B,H,N,M]
    out = (attn @ v per head, concat) @ Wo.T + bo  [B,N,D]

Sharding: core = b*4 + j (b in {0,1}; j owns heads 4j..4j+3 = a 256-wide
d-slice). v2 changes vs v1:

  - Mask compaction: the host packs only mask==1 key rows (plus zero pad
    to Mp, a multiple of 128). Attention + k/v projections shrink ~2x.
    Pad rows are killed by the -1e30 exp bias.
  - No collectives. The full-D rmsnorm sum-of-squares is computed locally
    per core via a host-side Cholesky factor: sumsq(x) = ||L^T x||^2 with
    G = W_raw^T W_raw = L L^T. L is block-lower-triangular, so only 36 of
    64 [128,128] blocks contribute. ss = colsum(z^2) via DVE square + PE
    ones-column matmul. This removes the AllReduce bootstrap (~110us) +
    latency (~70us) and the PE idle window that collapsed the HAM PE
    clock to 4/8 for the whole attention phase.
  - kfT streamed once: k-proj, z_k and v-proj all consume the same SBUF
    block. Input DMA spread over sync/gpsimd/scalar queues.
  - Out-projection interleaved into the attention instruction stream
    (one ot-chunk per m-tile) once an n-block's xT is finalized; only the
    last two n-blocks drain after attention.
"""

import numpy as np

import concourse.tile as tile
from concourse import bacc, mybir
from concourse.bass_utils import run_bass_kernel_spmd

B, N, M, D, H = 2, 2048, 2048, 1024, 16
C = D // H  # 64, head dim
E = D  # input feature dim
EPS = 1e-6
SCALE = C ** (-0.5)
DS = D // 4  # 256, per-core d-slice
NCORES = 8

f32 = mybir.dt.float32
f32r = mybir.dt.float32r
bf16 = mybir.dt.bfloat16
AF = mybir.ActivationFunctionType

NEG = -1e30

# block-lower-triangle of L in [e, zd] 128-blocks: L[e, zd] != 0 for zd <= e
TRI = [(dz, et) for dz in range(8) for et in range(dz, 8)]
NTRI = len(TRI)  # 36


def round_f32r(x: np.ndarray) -> np.ndarray:
    b = np.ascontiguousarray(x, dtype=np.float32).view(np.uint32)
    b = (b + 0x800) & np.uint32(0xFFFFF000)
    return b.view(np.float32)


def build(Mp: int):
    MT = Mp // 128
    # kf stream blocks of up to 512 columns
    W_LIST = [(s, min(512, Mp - s)) for s in range(0, Mp, 512)]

    nc = bacc.Bacc(None, target_bir_lowering=False)

    qT_d = nc.declare_dram_parameter("qT", [E, N], f32r, isOutput=False)
    kfT_d = nc.declare_dram_parameter("kfT", [E, Mp], f32r, isOutput=False)
    wqT_d = nc.declare_dram_parameter("wqT", [E, DS], f32r, isOutput=False)
    wkT_d = nc.declare_dram_parameter("wkT", [E, DS], f32r, isOutput=False)
    wvT_d = nc.declare_dram_parameter("wvT", [E, DS], f32r, isOutput=False)
    woT_d = nc.declare_dram_parameter("woT", [DS, D], f32r, isOutput=False)
    lq_d = nc.declare_dram_parameter("Lq", [128, NTRI * 128], f32r, isOutput=False)
    lk_d = nc.declare_dram_parameter("Lk", [128, NTRI * 128], f32r, isOutput=False)
    mb_d = nc.declare_dram_parameter("mbias", [MT, 128], f32, isOutput=False)
    outT_d = nc.declare_dram_parameter("outT", [D, N], bf16, isOutput=True)

    with (
        nc.allow_low_precision(reason="f32r matmul operands by design; fp32 PSUM"),
        tile.TileContext(nc) as tc,
    ):
        with (
            tc.tile_pool(name="singles", bufs=1) as singles,
            tc.tile_pool(name="wts", bufs=3) as wts,
            tc.tile_pool(name="lw", bufs=2) as lpool,
            tc.tile_pool(name="blk", bufs=3) as blkpool,
            tc.tile_pool(name="sq", bufs=2) as sqpool,
            tc.tile_pool(name="psb", bufs=3) as ppool,
            tc.tile_pool(name="obuf", bufs=2) as obuf,
            tc.tile_pool(name="osb", bufs=5) as osbp,
            tc.tile_pool(name="rdp", bufs=4) as rdp,
            tc.tile_pool(name="small", bufs=4) as small,
            tc.tile_pool(name="dram", bufs=1, space="DRAM") as dram,
        ):
            # round-robin input-stream DMA queues (PE untouched; DVE busy)
            inq = [nc.sync, nc.gpsimd, nc.scalar]
            qn = [0]

            def dq():
                e = inq[qn[0] % 3]
                qn[0] += 1
                return e

            outq_eng = [nc.sync, nc.scalar, nc.gpsimd]

            # ---- constants / small inputs ----
            ones_f = singles.tile([128, 64], f32)
            nc.vector.memset(ones_f, 1.0)
            ones1x64 = singles.tile([1, 64], f32)
            nc.vector.memset(ones1x64, 1.0)
            ones1x128 = singles.tile([1, 128], f32)
            nc.vector.memset(ones1x128, 1.0)
            ones_col = singles.tile([128, 1], f32r)
            nc.vector.tensor_copy(ones_col, ones_f[:, 0:1])
            eps_t = singles.tile([128, 1], f32)
            nc.vector.memset(eps_t, EPS)
            invd_t = singles.tile([128, 1], f32)
            nc.vector.memset(invd_t, 1.0 / D)
            mb_sb = singles.tile([128, MT], f32)
            nc.gpsimd.dma_start(out=mb_sb, in_=mb_d.rearrange("t p -> p t"))

            # ---- persistent activations ----
            qT = singles.tile([128, 2, 4, 512], f32r)  # [p, dt, nb, n]
            kT = singles.tile([128, 2, MT, 128], f32r)  # [p, dt, mt, m]
            v_sb = singles.tile([128, MT, 4, C + 1], bf16)  # [m_p, mt, h, c|ones]
            xT = qT  # aliased: each [h, nb] slice is written only after its last QK read
            nc.vector.tensor_copy(
                v_sb[:, :, :, C],
                ones_f[:, 0:MT * 4].rearrange("p (a b) -> p a b", a=MT),
            )
            ssq_row = singles.tile([1, 2048], f32)
            ssk_row = singles.tile([1, Mp], f32)

            # ---- weights (consumption order) ----
            wq_sb = wts.tile([128, 8, DS], f32r, tag="w")
            for et in range(8):
                dq().dma_start(out=wq_sb[:, et, :], in_=wqT_d[et * 128 : et * 128 + 128, :])
            lq_sb = lpool.tile([128, NTRI, 128], f32r, tag="L")
            for c in range(4):
                dq().dma_start(
                    out=lq_sb[:, c * 9 : c * 9 + 9, :],
                    in_=lq_d[:, c * 9 * 128 : (c * 9 + 9) * 128].rearrange(
                        "p (a b) -> p a b", a=9
                    ),
                )
            LIDX = {b: i for i, b in enumerate(TRI)}

            with (
                tc.tile_pool(name="projps", bufs=2, space="PSUM") as projps,
                tc.tile_pool(name="zps", bufs=2, space="PSUM") as zps,
                tc.tile_pool(name="ssps", bufs=2, space="PSUM") as ssps,
                tc.tile_pool(name="vps", bufs=2, space="PSUM") as vps,
            ):
                def emit_z(blk, l_sb, ss_seg, w):
                    """ss_seg[1, w] += colsum over full zd of (L^T x)^2."""
                    ss_ps = ssps.tile([1, 512], f32, tag="ss")
                    pend = []

                    def colsum(dz, zp):
                        sq = sqpool.tile([128, 512], f32r, tag="sq")
                        nc.scalar.activation(sq[:, 0:w], zp[:, 0:w], AF.Square)
                        nc.tensor.matmul(
                            ss_ps[:, 0:w],
                            ones_col,
                            sq[:, 0:w],
                            start=(dz == 0),
                            stop=(dz == 7),
                            skip_group_check=True,
                        )

                    for dz in range(8):
                        zp = zps.tile([128, 512], f32, tag="z")
                        for et in range(dz, 8):
                            nc.tensor.matmul(
                                zp[:, 0:w],
                                l_sb[:, LIDX[(dz, et)], :],
                                blk[:, et, 0:w],
                                start=(et == dz),
                                stop=(et == 7),
                            )
                        pend.append((dz, zp))
                        if len(pend) > 1:
                            colsum(*pend.pop(0))
                    colsum(*pend.pop(0))
                    nc.vector.tensor_copy(ss_seg, ss_ps[:, 0:w])

                # ---- P1: q projection + z_q, z staggered one block behind ----
                pend_z = []
                for nb in range(4):
                    blk = blkpool.tile([128, 8, 512], f32r, tag="blk")
                    for et in range(8):
                        dq().dma_start(
                            out=blk[:, et, :],
                            in_=qT_d[et * 128 : et * 128 + 128, nb * 512 : nb * 512 + 512],
                        )
                    for dt in range(2):
                        ps = projps.tile([128, 512], f32, tag="proj")
                        for et in range(8):
                            nc.tensor.matmul(
                                ps,
                                wq_sb[:, et, dt * 128 : dt * 128 + 128],
                                blk[:, et, :],
                                start=(et == 0),
                                stop=(et == 7),
                            )
                        nc.vector.tensor_copy(qT[:, dt, nb, :], ps)
                    pend_z.append((blk, nb))
                    if nb == 1:
                        # k weights + first kf block next in queue order
                        wk_sb = wts.tile([128, 8, DS], f32r, tag="w")
                        for et in range(8):
                            dq().dma_start(
                                out=wk_sb[:, et, :],
                                in_=wkT_d[et * 128 : et * 128 + 128, :],
                            )
                    if len(pend_z) > 1:
                        b0, n0 = pend_z.pop(0)
                        emit_z(b0, lq_sb, ssq_row[:, n0 * 512 : n0 * 512 + 512], 512)
                while pend_z:
                    b0, n0 = pend_z.pop(0)
                    emit_z(b0, lq_sb, ssq_row[:, n0 * 512 : n0 * 512 + 512], 512)

                # ---- rstd_q + qT finalize ----
                nc.scalar.activation(
                    ssq_row, ssq_row, AF.Sqrt, bias=eps_t[0:1, :], scale=invd_t[0:1, :]
                )
                rs_row = singles.tile([1, 2048], f32)
                nc.vector.reciprocal_approx_fast(out=rs_row, in_=ssq_row)
                for nb in range(4):
                    bcq = projps.tile([128, 512], f32, tag="proj")
                    nc.tensor.matmul(
                        bcq,
                        ones1x128,
                        rs_row[:, nb * 512 : nb * 512 + 512],
                        start=True,
                        stop=True,
                    )
                    for dt in range(2):
                        nc.vector.tensor_mul(qT[:, dt, nb, :], qT[:, dt, nb, :], bcq)

                # ---- P2: k proj + z_k + v proj, one kf stream ----
                lk_sb = lpool.tile([128, NTRI, 128], f32r, tag="L")
                wv_sb = wts.tile([128, 8, DS], f32r, tag="w")
                for bi, (s0, w) in enumerate(W_LIST):
                    blk = blkpool.tile([128, 8, 512], f32r, tag="blk")
                    for et in range(8):
                        dq().dma_start(
                            out=blk[:, et, 0:w],
                            in_=kfT_d[et * 128 : et * 128 + 128, s0 : s0 + w],
                        )
                    if bi == 0:
                        for c in range(4):
                            dq().dma_start(
                                out=lk_sb[:, c * 9 : c * 9 + 9, :],
                                in_=lk_d[:, c * 9 * 128 : (c * 9 + 9) * 128].rearrange(
                                    "p (a b) -> p a b", a=9
                                ),
                            )
                        for et in range(8):
                            dq().dma_start(
                                out=wv_sb[:, et, :],
                                in_=wvT_d[et * 128 : et * 128 + 128, :],
                            )
                    nmt = w // 128
                    for dt in range(2):
                        ps = projps.tile([128, 512], f32, tag="proj")
                        for et in range(8):
                            nc.tensor.matmul(
                                ps[:, 0:w],
                                wk_sb[:, et, dt * 128 : dt * 128 + 128],
                                blk[:, et, 0:w],
                                start=(et == 0),
                                stop=(et == 7),
                            )
                        nc.vector.tensor_copy(
                            kT[:, dt, bi * 4 : bi * 4 + nmt, :],
                            ps[:, 0:w].rearrange("p (a b) -> p a b", a=nmt),
                        )
                    emit_z(blk, lk_sb, ssk_row[:, s0 : s0 + w], w)
                    for mtL in range(nmt):
                        psv = vps.tile([128, 256], f32, tag="v")
                        for et in range(8):
                            nc.tensor.matmul(
                                psv,
                                blk[:, et, mtL * 128 : mtL * 128 + 128],
                                wv_sb[:, et, :],
                                start=(et == 0),
                                stop=(et == 7),
                            )
                        nc.vector.tensor_copy(
                            v_sb[:, bi * 4 + mtL, :, 0:C],
                            psv.rearrange("p (h c) -> p h c", c=C),
                        )

                # wo load (consumed mid-attention)
                wo_sb = wts.tile([128, 2, D], f32r, tag="w")
                for dc in range(2):
                    dq().dma_start(
                        out=wo_sb[:, dc, :], in_=woT_d[dc * 128 : dc * 128 + 128, :]
                    )

                # ---- rstd_k in row layout, folded into kT via bcast muls ----
                nc.scalar.activation(
                    ssk_row, ssk_row, AF.Sqrt, bias=eps_t[0:1, :], scale=invd_t[0:1, :]
                )
                rk_row = singles.tile([1, Mp], f32)
                nc.vector.reciprocal_approx_fast(out=rk_row, in_=ssk_row)
                for mt in range(MT):
                    bck = projps.tile([128, 512], f32, tag="proj")
                    nc.tensor.matmul(
                        bck[:, 0:128],
                        ones1x128,
                        rk_row[:, mt * 128 : mt * 128 + 128],
                        start=True,
                        stop=True,
                    )
                    for dt in range(2):
                        nc.vector.tensor_mul(
                            kT[:, dt, mt, :], kT[:, dt, mt, :], bck[:, 0:128]
                        )

            # ---- P4: attention, nbp-outer, out-proj interleaved ----
            with (
                tc.tile_pool(name="sps", bufs=3, space="PSUM") as spool,
                tc.tile_pool(name="ops", bufs=1, space="PSUM") as opool,
            ):
                def emit_outproj_chunk(nb, ot):
                    ps2 = spool.tile([128, 2, 512], f32, tag="s2")
                    ps = ps2[:, 0, :]
                    for dc in range(2):
                        nc.tensor.matmul(
                            ps,
                            wo_sb[:, dc, ot * 128 : ot * 128 + 128],
                            xT[:, dc, nb, :],
                            start=(dc == 0),
                            stop=(dc == 1),
                            skip_group_check=True,
                        )
                    out_sb = osbp.tile([128, 512], bf16, tag="osb")
                    nc.vector.tensor_copy(out_sb, ps)
                    outq_eng[(nb * 8 + ot) % 3].dma_start(
                        out=outT_d[ot * 128 : ot * 128 + 128, nb * 512 : nb * 512 + 512],
                        in_=out_sb,
                    )

                def emit_normalize(state):
                    """bc outer-products + muls for a pass whose DVE recips are
                    done by now (emitted one pass late to keep PE gapless)."""
                    hh, nbp, oo_sb, rds = state
                    ddt, ooff = hh // 2, (hh % 2) * C
                    for i, nb in enumerate((2 * nbp, 2 * nbp + 1)):
                        bc = spool.tile([128, 2, 512], f32, tag="s2")
                        nc.tensor.matmul(
                            bc[0:C, 0, :], ones1x64, rds[i], start=True, stop=True
                        )
                        nc.vector.tensor_mul(
                            xT[ooff : ooff + C, ddt, nb, :],
                            oo_sb[0:C, i, :],
                            bc[0:C, 0, :],
                        )

                prev = None
                for nbp in range(2):
                    for h in range(4):
                        dt, off = h // 2, (h % 2) * C
                        nbs = (2 * nbp, 2 * nbp + 1)
                        o2 = opool.tile([C + 1, 2, 512], f32, tag="o2")
                        for mt in range(MT):
                            kT_lhs = kT[off : off + C, dt, mt, :]
                            s2 = spool.tile([128, 2, 512], f32, tag="s2")
                            for i, nb in enumerate(nbs):
                                nc.tensor.matmul(
                                    s2[:, i, :],
                                    kT_lhs,
                                    qT[off : off + C, dt, nb, :],
                                    start=True,
                                    stop=True,
                                )
                            p2 = ppool.tile([128, 2, 512], bf16, tag="p")
                            nc.scalar.activation(
                                p2, s2, AF.Exp, bias=mb_sb[:, mt : mt + 1]
                            )
                            for i in range(2):
                                nc.tensor.matmul(
                                    o2[:, i, :],
                                    v_sb[:, mt, h, :],
                                    p2[:, i, :],
                                    start=(mt == 0),
                                    stop=(mt == MT - 1),
                                    skip_group_check=True,
                                )
                        o_sb = obuf.tile([C + 1, 2, 512], f32, tag="osb")
                        nc.vector.tensor_copy(o_sb, o2)
                        rds = []
                        for i in range(2):
                            den_sb = rdp.tile([1, 512], f32, tag="den")
                            nc.vector.tensor_copy(den_sb, o_sb[C : C + 1, i, :])
                            rd = rdp.tile([1, 512], f32, tag="rd")
                            nc.vector.reciprocal_approx_fast(out=rd, in_=den_sb)
                            rds.append(rd)
                        if prev is not None:
                            emit_normalize(prev)
                        prev = (h, nbp, o_sb, rds)
                emit_normalize(prev)
                for nb in range(4):
                    for ot in range(8):
                        emit_outproj_chunk(nb, ot)

    nc.finalize()
    return nc


_NC_CACHE = {}


def _get_nc(Mp=1024):
    if Mp not in _NC_CACHE:
        _NC_CACHE[Mp] = build(Mp)
    return _NC_CACHE[Mp]


def _chol_factor(W):
    G = W.astype(np.float64).T @ W.astype(np.float64)
    G += np.eye(E) * (1e-12 * np.trace(G) / E)
    L = np.linalg.cholesky(G)
    return L.astype(np.float32)  # [e, zd] lower


def _pack_L(L):
    P = np.empty((128, NTRI * 128), np.float32)
    for bi, (dz, et) in enumerate(TRI):
        P[:, bi * 128 : (bi + 1) * 128] = L[
            et * 128 : (et + 1) * 128, dz * 128 : (dz + 1) * 128
        ]
    return round_f32r(P)


def plan_Mp(mask):
    mask = np.asarray(mask)
    Mv = [int((mask[b] != 0).sum()) for b in range(B)]
    Mp = max(128, int(-(-max(max(Mv), 1) // 128)) * 128)
    return Mv, Mp


def make_in_maps(querys, key_feats, mask, Wq, Wk, Wv, gq, gk, Wo, bo):
    querys = np.asarray(querys, dtype=np.float32)
    key_feats = np.asarray(key_feats, dtype=np.float32)
    mask = np.asarray(mask)
    gq = np.asarray(gq, dtype=np.float32)
    gk = np.asarray(gk, dtype=np.float32)
    Wq = np.asarray(Wq, dtype=np.float32)
    Wk = np.asarray(Wk, dtype=np.float32)

    Mv, Mp = plan_Mp(mask)
    MT = Mp // 128

    gsq_full = gq * np.float32(SCALE)
    Wq_f = Wq * gsq_full[:, None]
    Wk_f = Wk * gk[:, None]
    lq_p = _pack_L(_chol_factor(Wq))
    lk_p = _pack_L(_chol_factor(Wk))

    qT, kfT, mb = [], [], []
    for b in range(B):
        idx = np.nonzero(mask[b])[0]
        kfc = np.zeros((Mp, E), np.float32)
        kfc[: len(idx)] = key_feats[b][idx]
        mbias = np.full((MT, 128), np.float32(NEG), np.float32)
        mbias.reshape(-1)[: len(idx)] = 0.0
        qT.append(round_f32r(querys[b].T))
        kfT.append(round_f32r(kfc.T))
        mb.append(mbias)

    wqT, wkT, wvT, woT = [], [], [], []
    for j in range(4):
        dsl = slice(j * DS, (j + 1) * DS)
        wqT.append(round_f32r(Wq_f[dsl].T))
        wkT.append(round_f32r(Wk_f[dsl].T))
        wvT.append(round_f32r(np.asarray(Wv)[dsl].T))
        woT.append(round_f32r(np.asarray(Wo)[:, dsl].T))

    in_maps = []
    for cid in range(NCORES):
        b, j = cid // 4, cid % 4
        in_maps.append(
            {
                "qT": qT[b],
                "kfT": kfT[b],
                "wqT": wqT[j],
                "wkT": wkT[j],
                "wvT": wvT[j],
                "woT": woT[j],
                "Lq": lq_p,
                "Lk": lk_p,
                "mbias": mb[b],
            }
        )
    return in_maps


def assemble(results, mask, bo):
    mask = np.asarray(mask)
    bo = np.asarray(bo, dtype=np.float32)
    out = np.zeros((B, N, D), dtype=np.float32)
    for cid in range(NCORES):
        b = cid // 4
        out[b] += np.asarray(results[cid]["outT"]).astype(np.float32).T
    out += bo
    for b in range(B):
        if (mask[b] != 0).sum() == 0:
            out[b] = bo  # reference: all-masked row -> attn = 0
    return out


def kernel(querys, key_feats, mask, Wq, Wk, Wv, gq, gk, Wo, bo):
    _, Mp = plan_Mp(mask)
    nc = _get_nc(Mp)
    in_maps = make_in_maps(querys, key_feats, mask, Wq, Wk, Wv, gq, gk, Wo, bo)
    res = run_bass_kernel_spmd(nc, in_maps, list(range(NCORES)))
    return assemble(res.results, mask, bo)
